# revision 2
# baseline (speedup 1.0000x reference)
"""Fused GPTQ-style dequant + GEMM kernel for 8 TRN2 NeuronCores.

Reference computation (per problem):
    w = (q - zp[g]) * scale[g]   per group g of 128 consecutive k values
    out = active @ w + bias      active [256, 4096], w [4096, 11008]

Sharding: tensor-parallel along N (output features). Each of 8 cores gets
an 11008/8 = 1376-wide slice of weight/scale/zp/bias; activations are
replicated; outputs concatenated on host.

Device algorithm (per core):
    out = aT.T @ (q * scale_bcast)  +  (-r).T @ (zp*scale)  +  1 x bias
  - weights are shipped as bf16 (codes 0..15 are exact in bf16) and
    dequantized on VectorE as q*scale with a partition-broadcast scale
    (materialized once via log-doubling SBUF->SBUF DMAs).
  - the zero-point term factors into a rank-32 correction GEMM:
    sum_g r_g[s] * (zp*scale)[g,n] with r_g = per-group row-sums of the
    activations, computed on TensorE with a block one-hot (-I) operand.
  - bias rides the same correction GEMM as a 33rd contraction row.
  - All 32 group matmuls + correction accumulate into the same PSUM banks.
"""

import sys

sys.path.insert(0, "/opt/trn_rl_repo")

import numpy as np
import ml_dtypes

import concourse.bass as bass
import concourse.bacc as bacc
import concourse.mybir as mybir
import concourse.tile as tile
from concourse.bass import ts, ds

BF16 = mybir.dt.bfloat16
F32 = mybir.dt.float32

P = 128           # partitions / group size
G = 32            # quant groups
K = 4096          # contraction dim
S = 256           # sequence (rows of activation)
N_FULL = 11008
NCORES = 8
NSH = N_FULL // NCORES      # 1376 output features per core
GCHUNK = 4                  # weight groups per DMA/dequant chunk
N_SPLITS = (512, 512, 352)  # psum free-dim chunking of NSH

_NC_CACHE = {}


def build_nc():
    """Build the single-core Bass graph (same graph runs SPMD on all 8 cores)."""
    nc = bacc.Bacc(None)

    # DRAM parameters (inputs per core)
    aT_d = nc.declare_dram_parameter("aT", [K, S], F32, isOutput=False)
    wq_d = nc.declare_dram_parameter("wq", [G, P, NSH], BF16, isOutput=False)
    sc_d = nc.declare_dram_parameter("scale", [G, NSH], F32, isOutput=False)
    zp_d = nc.declare_dram_parameter("zp", [G, NSH], F32, isOutput=False)
    bias_d = nc.declare_dram_parameter("biasr", [1, NSH], F32, isOutput=False)
    eneg_d = nc.declare_dram_parameter("eneg", [P, G, G], BF16, isOutput=False)
    out_d = nc.declare_dram_parameter("out", [S, NSH], F32, isOutput=True)

    n_off = [0, 512, 1024]

    with tile.TileContext(nc) as tc:
        with (
            tc.tile_pool(name="const", bufs=1) as const,
            tc.tile_pool(name="wpool", bufs=3) as wpool,
            tc.tile_pool(name="psum", bufs=1, space="PSUM") as psum,
        ):
            # ---------------- preamble ----------------
            aT_f = const.tile([P, G, S], F32)       # 32 KB/part
            nc.sync.dma_start(aT_f[:], aT_d.rearrange("(g p) s -> p g s", p=P))
            aT = const.tile([P, G, S], BF16)        # 16 KB/part
            nc.scalar.copy(aT[:], aT_f[:])

            eneg = const.tile([P, G, G], BF16)
            nc.sync.dma_start(eneg[:], eneg_d[:])

            sc_f = const.tile([G, NSH], F32)
            zp_f = const.tile([G, NSH], F32)
            nc.sync.dma_start(sc_f[:], sc_d[:])
            nc.sync.dma_start(zp_f[:], zp_d[:])

            # correction rhs: rows 0..31 = zp*scale, row 32 = bias, 33..63 = 0
            corr_rhs = const.tile([64, NSH], BF16)
            nc.vector.memset(corr_rhs[:], 0.0)
            nc.vector.tensor_tensor(
                corr_rhs[0:G, :], zp_f[:], sc_f[:], mybir.AluOpType.mult
            )
            bias_f = const.tile([1, NSH], F32)
            nc.sync.dma_start(bias_f[:], bias_d[:])
            bias_bf = const.tile([1, NSH], BF16)
            nc.scalar.copy(bias_bf[:], bias_f[:])
            # engines can't cross partitions: move row via DMA
            nc.sync.dma_start(corr_rhs[G : G + 1, :], bias_bf[:])

            # broadcast scale to all 128 partitions (bf16), via log-doubling
            sc_rows = const.tile([G, NSH], BF16)
            nc.vector.tensor_copy(sc_rows[:], sc_f[:])
            sc_b = const.tile([P, G, NSH], BF16)    # 88 KB/part
            # gather the 32 rows into partition 0's 32 blocks
            nc.sync.dma_start(sc_b[0:1, :, :], sc_rows[:])
            p2 = 1
            while p2 < P:
                nc.sync.dma_start(sc_b[p2 : 2 * p2], sc_b[0:p2])
                p2 *= 2

            # psum accumulators: [2 s-chunks][3 n-chunks] + r accumulator
            acc = [
                [psum.tile([P, nw], F32, name=f"acc_{si}_{nj}") for nj, nw in enumerate(N_SPLITS)]
                for si in range(2)
            ]
            psum_r = psum.tile([G, S], F32, name="psum_r")

            # ---------------- main loop ----------------
            nchunks = G // GCHUNK
            for c in range(nchunks):
                g0 = c * GCHUNK
                wq = wpool.tile([P, GCHUNK, NSH], BF16, tag="wq")
                nc.scalar.dma_start(
                    wq[:], wq_d[g0 : g0 + GCHUNK].rearrange("g p n -> p g n")
                )
                # dequant in place: w *= scale (broadcast along partitions)
                nc.vector.tensor_tensor(
                    wq[:], wq[:], sc_b[:, g0 : g0 + GCHUNK, :], mybir.AluOpType.mult
                )
                for gl in range(GCHUNK):
                    g = g0 + gl
                    # r accumulation: psum_r[j,s] += sum_p eneg[p,g,j]*aT[p,g,s]
                    nc.tensor.matmul(
                        psum_r[:],
                        eneg[:, g, :],
                        aT[:, g, :],
                        start=(g == 0),
                        stop=(g == G - 1),
                    )
                    for si in range(2):
                        lhsT = aT[:, g, ts(si, P)]
                        for nj, nw in enumerate(N_SPLITS):
                            nc.tensor.matmul(
                                acc[si][nj][:, :nw],
                                lhsT,
                                wq[:, gl, ds(n_off[nj], nw)],
                                start=(g == 0),
                                stop=False,
                            )

            # ---------------- correction + epilogue ----------------
            corr_lhsT = const.tile([64, S], BF16)
            nc.vector.memset(corr_lhsT[:], 0.0)
            nc.vector.tensor_copy(corr_lhsT[0:G, :], psum_r[:])
            nc.vector.memset(corr_lhsT[G : G + 1, :], 1.0)

            out_sb = const.tile([P, 2, NSH], F32)   # 11 KB/part
            for si in range(2):
                for nj, nw in enumerate(N_SPLITS):
                    nc.tensor.matmul(
                        acc[si][nj][:, :nw],
                        corr_lhsT[:, ts(si, P)],
                        corr_rhs[:, ds(n_off[nj], nw)],
                        start=False,
                        stop=True,
                    )
                    eng = nc.scalar if (si + nj) % 2 else nc.vector
                    if eng is nc.scalar:
                        nc.scalar.copy(
                            out_sb[:, si, ds(n_off[nj], nw)], acc[si][nj][:, :nw]
                        )
                    else:
                        nc.vector.tensor_copy(
                            out_sb[:, si, ds(n_off[nj], nw)], acc[si][nj][:, :nw]
                        )

            nc.sync.dma_start(out_d.rearrange("(so p) n -> p so n", p=P), out_sb[:])

    nc.compile()
    return nc


def _prep_in_maps(active, weight, scale, zp, bias):
    a2 = np.ascontiguousarray(
        np.asarray(active, dtype=np.float32).reshape(S, K).T
    )  # aT [K, S] f32
    wq_bf = np.asarray(weight).astype(ml_dtypes.bfloat16)  # codes 0..15, exact
    scale = np.asarray(scale, dtype=np.float32)
    zp = np.asarray(zp, dtype=np.float32)
    bias = np.asarray(bias, dtype=np.float32)

    # block one-hot (negated identity broadcast to 128 partitions)
    eneg = np.broadcast_to(
        -np.eye(G, dtype=ml_dtypes.bfloat16), (P, G, G)
    ).copy()

    in_maps = []
    for i in range(NCORES):
        sl = slice(i * NSH, (i + 1) * NSH)
        in_maps.append(
            {
                "aT": a2,
                "wq": np.ascontiguousarray(wq_bf[:, :, sl]),
                "scale": np.ascontiguousarray(scale[:, sl]),
                "zp": np.ascontiguousarray(zp[:, sl]),
                "biasr": np.ascontiguousarray(bias[sl]).reshape(1, NSH),
                "eneg": eneg,
            }
        )
    return in_maps


def run_on_hw(inputs, trace=False):
    """Run the SPMD kernel; returns (full_output, BassKernelResults)."""
    from concourse.bass_utils import run_bass_kernel_spmd

    if "nc" not in _NC_CACHE:
        _NC_CACHE["nc"] = build_nc()
    nc = _NC_CACHE["nc"]
    in_maps = _prep_in_maps(
        inputs["active"], inputs["weight"], inputs["scale"],
        inputs["zp"], inputs["bias"],
    )
    res = run_bass_kernel_spmd(
        nc, in_maps, core_ids=list(range(NCORES)), trace=trace
    )
    parts = [np.asarray(res.results[i]["out"]) for i in range(NCORES)]
    full = np.concatenate(parts, axis=-1).reshape(1, 1, S, N_FULL)
    return np.ascontiguousarray(full, dtype=np.float32), res


def kernel(**inputs) -> np.ndarray:
    assert int(inputs.get("group_size", P)) == P
    assert int(inputs.get("weight_bits", 4)) == 4
    out, _ = run_on_hw(inputs, trace=False)
    return out


# revision 4
# speedup vs baseline: 1.3905x; 1.3905x over previous
"""Fused GPTQ-style dequant + GEMM kernel for 8 TRN2 NeuronCores.

Reference computation (per problem):
    w = (q - zp[g]) * scale[g]   per group g of 128 consecutive k values
    out = active @ w + bias      active [256, 4096], w [4096, 11008]

Sharding: tensor-parallel along N (output features). Each of 8 cores gets
an 11008/8 = 1376-wide slice of weight/scale/zp/bias; activations are
replicated; outputs concatenated on host.

Device algorithm (per core):
    out = aT.T @ (q * scale_bcast)  +  (-r).T @ (zp*scale)  +  1 x bias
  - weights shipped as bf16 codes (0..15 exact), partition-major for big
    DMA descriptors; dequantized on VectorE as q*scale.
  - scale broadcast to all 128 partitions per 4-group chunk: one SWDGE
    cast-DMA from DRAM with a stride-0 32x replication, then two
    contiguous SBUF->SBUF doubling fanouts (32->64, 64->128).
  - zero-point term folds into a rank-32 correction GEMM via per-group
    activation row-sums r (computed on TensorE with -onehot blocks);
    bias rides the same correction GEMM as a 33rd row.
  - All 32 group matmuls + correction accumulate in the same PSUM banks.
"""

import sys

sys.path.insert(0, "/opt/trn_rl_repo")

import numpy as np
import ml_dtypes

import concourse.bass as bass
import concourse.bacc as bacc
import concourse.mybir as mybir
import concourse.tile as tile
from concourse.bass import ts, ds

BF16 = mybir.dt.bfloat16
F32 = mybir.dt.float32

P = 128           # partitions / group size
G = 32            # quant groups
K = 4096          # contraction dim
S = 256           # sequence (rows of activation)
N_FULL = 11008
NCORES = 8
NSH = N_FULL // NCORES      # 1376 output features per core
GCHUNK = 4                  # weight groups per DMA/dequant chunk
NCHUNKS = G // GCHUNK
ATCH = 8                    # groups per activation slice-tile
N_SPLITS = (512, 512, 352)  # psum free-dim chunking of NSH

_NC_CACHE = {}


def build_nc():
    """Build the single-core Bass graph (same graph runs SPMD on all 8 cores)."""
    nc = bacc.Bacc(None)

    aT_d = nc.declare_dram_parameter("aT", [P, G, S], F32, isOutput=False)
    wq_d = nc.declare_dram_parameter("wq", [P, G, NSH], BF16, isOutput=False)
    sc_d = nc.declare_dram_parameter("scale", [G, NSH], F32, isOutput=False)
    zp_d = nc.declare_dram_parameter("zp", [G, NSH], F32, isOutput=False)
    bias_d = nc.declare_dram_parameter("biasr", [1, NSH], F32, isOutput=False)
    eneg_d = nc.declare_dram_parameter("eneg", [P, G, G], BF16, isOutput=False)
    out_d = nc.declare_dram_parameter("out", [S, NSH], F32, isOutput=True)

    n_off = [0, 512, 1024]

    with tile.TileContext(nc) as tc:
        with (
            tc.tile_pool(name="const", bufs=1) as const,
            tc.tile_pool(name="wpool", bufs=3) as wpool,
            tc.tile_pool(name="psum", bufs=1, space="PSUM") as psum,
        ):
            # ---------------- preamble ----------------
            # activations: partition-major f32 in DRAM, SWDGE cast to bf16,
            # in 4 slice-tiles so matmuls can start before the full 4MB lands
            aT = []
            for q in range(G // ATCH):
                t = const.tile([P, ATCH, S], BF16, name=f"aT{q}")
                nc.gpsimd.dma_start(t[:], aT_d[:, ts(q, ATCH), :])
                aT.append(t)

            eneg = const.tile([P, G, G], BF16)
            nc.sync.dma_start(eneg[:], eneg_d[:])

            sc_f = const.tile([G, NSH], F32)
            zp_f = const.tile([G, NSH], F32)
            nc.sync.dma_start(sc_f[:], sc_d[:])
            nc.sync.dma_start(zp_f[:], zp_d[:])

            # correction rhs: rows 0..31 = zp*scale, row 32 = bias, 33..63 = 0
            corr_rhs = const.tile([64, NSH], BF16)
            nc.vector.memset(corr_rhs[:], 0.0)
            nc.vector.tensor_tensor(
                corr_rhs[0:G, :], zp_f[:], sc_f[:], mybir.AluOpType.mult
            )
            bias_f = const.tile([1, NSH], F32)
            nc.sync.dma_start(bias_f[:], bias_d[:])
            bias_bf = const.tile([1, NSH], BF16)
            nc.scalar.copy(bias_bf[:], bias_f[:])
            nc.sync.dma_start(corr_rhs[G : G + 1, :], bias_bf[:])

            # per-chunk broadcast scale tiles: seed 32 partitions straight
            # from DRAM (stride-0 replicated read + f32->bf16 cast on SWDGE),
            # then two contiguous doubling fanouts on the sync ring.
            scb = []
            for c in range(NCHUNKS):
                t = const.tile([P, GCHUNK, NSH], BF16, name=f"scb{c}")
                src = sc_d[None, ts(c, GCHUNK), :].to_broadcast([32, GCHUNK, NSH])
                nc.gpsimd.dma_start(t[0:32], src)
                nc.sync.dma_start(t[32:64], t[0:32])
                nc.sync.dma_start(t[64:128], t[0:64])
                scb.append(t)

            # psum accumulators: [2 s-chunks][3 n-chunks] + r accumulator
            acc = [
                [psum.tile([P, nw], F32, name=f"acc_{si}_{nj}") for nj, nw in enumerate(N_SPLITS)]
                for si in range(2)
            ]
            psum_r = psum.tile([G, S], F32, name="psum_r")

            # ---------------- main loop ----------------
            for c in range(NCHUNKS):
                g0 = c * GCHUNK
                wq = wpool.tile([P, GCHUNK, NSH], BF16, tag="wq")
                nc.scalar.dma_start(wq[:], wq_d[:, ts(c, GCHUNK), :])
                # dequant in place: w *= scale (partition-broadcast tile)
                nc.vector.tensor_tensor(
                    wq[:], wq[:], scb[c][:], mybir.AluOpType.mult
                )
                for gl in range(GCHUNK):
                    g = g0 + gl
                    a_g = aT[g // ATCH][:, g % ATCH, :]
                    # r accumulation: psum_r[j,s] += sum_p eneg[p,g,j]*aT[p,g,s]
                    nc.tensor.matmul(
                        psum_r[:],
                        eneg[:, g, :],
                        a_g,
                        start=(g == 0),
                        stop=(g == G - 1),
                    )
                    for si in range(2):
                        lhsT = a_g[:, ts(si, P)]
                        for nj, nw in enumerate(N_SPLITS):
                            nc.tensor.matmul(
                                acc[si][nj][:, :nw],
                                lhsT,
                                wq[:, gl, ds(n_off[nj], nw)],
                                start=(g == 0),
                                stop=False,
                            )

            # ---------------- correction + epilogue ----------------
            corr_lhsT = const.tile([64, S], BF16)
            nc.vector.memset(corr_lhsT[:], 0.0)
            nc.vector.tensor_copy(corr_lhsT[0:G, :], psum_r[:])
            nc.vector.memset(corr_lhsT[G : G + 1, :], 1.0)

            out_sb = const.tile([P, 2, NSH], F32)   # 11 KB/part
            for si in range(2):
                for nj, nw in enumerate(N_SPLITS):
                    nc.tensor.matmul(
                        acc[si][nj][:, :nw],
                        corr_lhsT[:, ts(si, P)],
                        corr_rhs[:, ds(n_off[nj], nw)],
                        start=False,
                        stop=True,
                    )
                    if (si + nj) % 2:
                        nc.scalar.copy(
                            out_sb[:, si, ds(n_off[nj], nw)], acc[si][nj][:, :nw]
                        )
                    else:
                        nc.vector.tensor_copy(
                            out_sb[:, si, ds(n_off[nj], nw)], acc[si][nj][:, :nw]
                        )

            nc.sync.dma_start(out_d.rearrange("(so p) n -> p so n", p=P), out_sb[:])

    nc.compile()
    return nc


def _prep_in_maps(active, weight, scale, zp, bias):
    a2 = np.asarray(active, dtype=np.float32).reshape(S, K)
    # aT partition-major: [P, G, S] where k = g*128 + p
    aTp = np.ascontiguousarray(a2.T.reshape(G, P, S).transpose(1, 0, 2))
    wq_bf = np.asarray(weight).astype(ml_dtypes.bfloat16)  # codes 0..15, exact
    scale = np.asarray(scale, dtype=np.float32)
    zp = np.asarray(zp, dtype=np.float32)
    bias = np.asarray(bias, dtype=np.float32)

    eneg = np.broadcast_to(
        -np.eye(G, dtype=ml_dtypes.bfloat16), (P, G, G)
    ).copy()

    in_maps = []
    for i in range(NCORES):
        sl = slice(i * NSH, (i + 1) * NSH)
        in_maps.append(
            {
                "aT": aTp,
                # weight [G, P, nsh] -> partition-major [P, G, nsh]
                "wq": np.ascontiguousarray(wq_bf[:, :, sl].transpose(1, 0, 2)),
                "scale": np.ascontiguousarray(scale[:, sl]),
                "zp": np.ascontiguousarray(zp[:, sl]),
                "biasr": np.ascontiguousarray(bias[sl]).reshape(1, NSH),
                "eneg": eneg,
            }
        )
    return in_maps


def run_on_hw(inputs, trace=False):
    """Run the SPMD kernel; returns (full_output, BassKernelResults)."""
    from concourse.bass_utils import run_bass_kernel_spmd

    if "nc" not in _NC_CACHE:
        _NC_CACHE["nc"] = build_nc()
    nc = _NC_CACHE["nc"]
    in_maps = _prep_in_maps(
        inputs["active"], inputs["weight"], inputs["scale"],
        inputs["zp"], inputs["bias"],
    )
    res = run_bass_kernel_spmd(
        nc, in_maps, core_ids=list(range(NCORES)), trace=trace
    )
    parts = [np.asarray(res.results[i]["out"]) for i in range(NCORES)]
    full = np.concatenate(parts, axis=-1).reshape(1, 1, S, N_FULL)
    return np.ascontiguousarray(full, dtype=np.float32), res


def kernel(**inputs) -> np.ndarray:
    assert int(inputs.get("group_size", P)) == P
    assert int(inputs.get("weight_bits", 4)) == 4
    out, _ = run_on_hw(inputs, trace=False)
    return out


# revision 6
# speedup vs baseline: 1.4978x; 1.0772x over previous
"""Fused GPTQ-style dequant + GEMM kernel for 8 TRN2 NeuronCores.

Reference computation (per problem):
    w = (q - zp[g]) * scale[g]   per group g of 128 consecutive k values
    out = active @ w + bias      active [256, 4096], w [4096, 11008]

Sharding: tensor-parallel along N (output features). Each of 8 cores gets
an 11008/8 = 1376-wide slice of weight/scale/zp/bias; activations are
replicated; outputs concatenated on host.

Device algorithm (per core):
    out = aT.T @ (q * scale_bcast)  +  (-r).T @ (zp*scale)  +  1 x bias
  - weights shipped as bf16 codes (0..15 exact), partition-major for big
    DMA descriptors; dequantized on VectorE as q*scale.
  - scale broadcast to all 128 partitions per 4-group chunk: one SWDGE
    cast-DMA from DRAM with a stride-0 32x replication, then two
    contiguous SBUF->SBUF doubling fanouts (32->64, 64->128).
  - zero-point term folds into a rank-32 correction GEMM via per-group
    activation row-sums r (computed on TensorE with -onehot blocks);
    bias rides the same correction GEMM as a 33rd row.
  - All 32 group matmuls + correction accumulate in the same PSUM banks.
"""

import sys

sys.path.insert(0, "/opt/trn_rl_repo")

import numpy as np
import ml_dtypes

import concourse.bass as bass
import concourse.bacc as bacc
import concourse.mybir as mybir
import concourse.tile as tile
from concourse.bass import ts, ds

BF16 = mybir.dt.bfloat16
F32 = mybir.dt.float32

P = 128           # partitions / group size
G = 32            # quant groups
K = 4096          # contraction dim
S = 256           # sequence (rows of activation)
N_FULL = 11008
NCORES = 8
NSH = N_FULL // NCORES      # 1376 output features per core
GCHUNK = 4                  # weight groups per DMA/dequant chunk
NCHUNKS = G // GCHUNK
ATCH = 16                   # groups per activation slice-tile
N_SPLITS = (512, 512, 352)  # psum free-dim chunking of NSH

_NC_CACHE = {}


def build_nc():
    """Build the single-core Bass graph (same graph runs SPMD on all 8 cores)."""
    nc = bacc.Bacc(None)

    aT_d = nc.declare_dram_parameter("aT", [P, G, S], F32, isOutput=False)
    wq_d = nc.declare_dram_parameter("wq", [P, G, NSH], BF16, isOutput=False)
    sc_d = nc.declare_dram_parameter("scale", [G, NSH], F32, isOutput=False)
    zp_d = nc.declare_dram_parameter("zp", [G, NSH], F32, isOutput=False)
    bias_d = nc.declare_dram_parameter("biasr", [1, NSH], F32, isOutput=False)
    eneg_d = nc.declare_dram_parameter("eneg", [P, G, G], BF16, isOutput=False)
    out_d = nc.declare_dram_parameter("out", [S, NSH], F32, isOutput=True)

    n_off = [0, 512, 1024]

    with tile.TileContext(nc) as tc:
        with (
            tc.tile_pool(name="const", bufs=1) as const,
            tc.tile_pool(name="wpool", bufs=3) as wpool,
            tc.tile_pool(name="psum", bufs=1, space="PSUM") as psum,
        ):
            # ---------------- preamble ----------------
            # stage scale as bf16 in DRAM (one SWDGE cast) so all seed DMAs
            # can be cast-free HWDGE reads
            sc_bf_d = nc.dram_tensor("sc_bf_stage", [G, NSH], BF16, kind="Internal")
            nc.gpsimd.dma_start(sc_bf_d[:], sc_d[:])

            # activations: partition-major f32 in DRAM, SWDGE cast to bf16,
            # two halves so early matmuls don't wait for the full 4MB
            aT = []
            for q in range(G // ATCH):
                t = const.tile([P, ATCH, S], BF16, name=f"aT{q}")
                nc.gpsimd.dma_start(t[:], aT_d[:, ts(q, ATCH), :])
                aT.append(t)

            eneg = const.tile([P, G, G], BF16)
            nc.sync.dma_start(eneg[:], eneg_d[:])

            sc_f = const.tile([G, NSH], F32)
            zp_f = const.tile([G, NSH], F32)
            nc.sync.dma_start(sc_f[:], sc_d[:])
            nc.sync.dma_start(zp_f[:], zp_d[:])

            # correction rhs: rows 0..31 = zp*scale, row 32 = bias, 33..63 = 0
            corr_rhs = const.tile([64, NSH], BF16)
            nc.vector.memset(corr_rhs[:], 0.0)
            nc.vector.tensor_tensor(
                corr_rhs[0:G, :], zp_f[:], sc_f[:], mybir.AluOpType.mult
            )
            bias_f = const.tile([1, NSH], F32)
            nc.sync.dma_start(bias_f[:], bias_d[:])
            bias_bf = const.tile([1, NSH], BF16)
            nc.scalar.copy(bias_bf[:], bias_f[:])
            nc.sync.dma_start(corr_rhs[G : G + 1, :], bias_bf[:])

            # per-chunk broadcast scale tiles: seed 32 partitions from the
            # bf16 DRAM staging with a stride-0 32x replicated read (HWDGE),
            # then 3 independent contiguous fanouts (no dependency chains).
            scb = [
                const.tile([P, GCHUNK, NSH], BF16, name=f"scb{c}")
                for c in range(NCHUNKS)
            ]
            for c in range(NCHUNKS):
                src = sc_bf_d[None, ts(c, GCHUNK), :].to_broadcast(
                    [32, GCHUNK, NSH]
                )
                nc.sync.dma_start(scb[c][0:32], src)
            for c in range(NCHUNKS):
                ring = nc.sync if c % 2 == 0 else nc.scalar
                ring.dma_start(scb[c][32:64], scb[c][0:32])
                ring.dma_start(scb[c][64:96], scb[c][0:32])
                ring.dma_start(scb[c][96:128], scb[c][0:32])

            # psum accumulators: [2 s-chunks][3 n-chunks] + r accumulator
            acc = [
                [psum.tile([P, nw], F32, name=f"acc_{si}_{nj}") for nj, nw in enumerate(N_SPLITS)]
                for si in range(2)
            ]
            psum_r = psum.tile([G, S], F32, name="psum_r")

            # ---------------- main loop ----------------
            for c in range(NCHUNKS):
                g0 = c * GCHUNK
                wq = wpool.tile([P, GCHUNK, NSH], BF16, tag="wq")
                nc.scalar.dma_start(wq[:], wq_d[:, ts(c, GCHUNK), :])
                # dequant in place: w *= scale (partition-broadcast tile)
                nc.vector.tensor_tensor(
                    wq[:], wq[:], scb[c][:], mybir.AluOpType.mult
                )
                for gl in range(GCHUNK):
                    g = g0 + gl
                    a_g = aT[g // ATCH][:, g % ATCH, :]
                    # r accumulation: psum_r[j,s] += sum_p eneg[p,g,j]*aT[p,g,s]
                    nc.tensor.matmul(
                        psum_r[:],
                        eneg[:, g, :],
                        a_g,
                        start=(g == 0),
                        stop=(g == G - 1),
                    )
                    for si in range(2):
                        lhsT = a_g[:, ts(si, P)]
                        for nj, nw in enumerate(N_SPLITS):
                            nc.tensor.matmul(
                                acc[si][nj][:, :nw],
                                lhsT,
                                wq[:, gl, ds(n_off[nj], nw)],
                                start=(g == 0),
                                stop=False,
                            )

            # ---------------- correction + epilogue ----------------
            corr_lhsT = const.tile([64, S], BF16)
            nc.vector.memset(corr_lhsT[:], 0.0)
            nc.vector.tensor_copy(corr_lhsT[0:G, :], psum_r[:])
            nc.vector.memset(corr_lhsT[G : G + 1, :], 1.0)

            out_sb = const.tile([P, 2, NSH], F32)   # 11 KB/part
            for si in range(2):
                for nj, nw in enumerate(N_SPLITS):
                    nc.tensor.matmul(
                        acc[si][nj][:, :nw],
                        corr_lhsT[:, ts(si, P)],
                        corr_rhs[:, ds(n_off[nj], nw)],
                        start=False,
                        stop=True,
                    )
                    if (si + nj) % 2:
                        nc.scalar.copy(
                            out_sb[:, si, ds(n_off[nj], nw)], acc[si][nj][:, :nw]
                        )
                    else:
                        nc.vector.tensor_copy(
                            out_sb[:, si, ds(n_off[nj], nw)], acc[si][nj][:, :nw]
                        )

            nc.sync.dma_start(out_d.rearrange("(so p) n -> p so n", p=P), out_sb[:])

    nc.compile()
    return nc


def _prep_in_maps(active, weight, scale, zp, bias):
    a2 = np.asarray(active, dtype=np.float32).reshape(S, K)
    # aT partition-major: [P, G, S] where k = g*128 + p
    aTp = np.ascontiguousarray(a2.T.reshape(G, P, S).transpose(1, 0, 2))
    wq_bf = np.asarray(weight).astype(ml_dtypes.bfloat16)  # codes 0..15, exact
    scale = np.asarray(scale, dtype=np.float32)
    zp = np.asarray(zp, dtype=np.float32)
    bias = np.asarray(bias, dtype=np.float32)

    eneg = np.broadcast_to(
        -np.eye(G, dtype=ml_dtypes.bfloat16), (P, G, G)
    ).copy()

    in_maps = []
    for i in range(NCORES):
        sl = slice(i * NSH, (i + 1) * NSH)
        in_maps.append(
            {
                "aT": aTp,
                # weight [G, P, nsh] -> partition-major [P, G, nsh]
                "wq": np.ascontiguousarray(wq_bf[:, :, sl].transpose(1, 0, 2)),
                "scale": np.ascontiguousarray(scale[:, sl]),
                "zp": np.ascontiguousarray(zp[:, sl]),
                "biasr": np.ascontiguousarray(bias[sl]).reshape(1, NSH),
                "eneg": eneg,
            }
        )
    return in_maps


def run_on_hw(inputs, trace=False):
    """Run the SPMD kernel; returns (full_output, BassKernelResults)."""
    from concourse.bass_utils import run_bass_kernel_spmd

    if "nc" not in _NC_CACHE:
        _NC_CACHE["nc"] = build_nc()
    nc = _NC_CACHE["nc"]
    in_maps = _prep_in_maps(
        inputs["active"], inputs["weight"], inputs["scale"],
        inputs["zp"], inputs["bias"],
    )
    res = run_bass_kernel_spmd(
        nc, in_maps, core_ids=list(range(NCORES)), trace=trace
    )
    parts = [np.asarray(res.results[i]["out"]) for i in range(NCORES)]
    full = np.concatenate(parts, axis=-1).reshape(1, 1, S, N_FULL)
    return np.ascontiguousarray(full, dtype=np.float32), res


def kernel(**inputs) -> np.ndarray:
    assert int(inputs.get("group_size", P)) == P
    assert int(inputs.get("weight_bits", 4)) == 4
    out, _ = run_on_hw(inputs, trace=False)
    return out
